# revision 22
# baseline (speedup 1.0000x reference)
"""Multi-head causal self-attention on 8 Trainium2 NeuronCores.

Tensor-parallel over heads: core i owns heads (2i, 2i+1). bf16 matmul
operands throughout (fp32 PSUM accumulation); harness tolerance 2e-2.

Per core (v5 — deep-pipelined emission with filler thunks):
  One merged pipeline: attention for batch 0 starts right after the
  first 512-token QKV block, so the Exp activation engine (the
  attention bottleneck at ~1.1us per ki step) ramps early.  Remaining
  QKV chains and the projection matmuls are emitted as small "filler"
  thunks popped between ki iterations of later attention blocks — the
  per-engine queues are strict FIFO, so placement order is execution
  order, and every filler is placed late enough that its dependencies
  (DMAs, collectives) are already satisfied when it reaches the queue
  head.

  qkv: per 512-token block, q/k/v = (W_slice^T @ x^T); x is host-packed
       [128, g, nl, kc, 512] so each load is one DMA with 8KB
       contiguous lines (descriptor-count, not bytes, dominates
       dispatch cost). vT via PE transposes.
  attn: per (b, qj, ki): scoresT for both heads as two row-tiled K=64
       matmuls concurrent on PE rows 0-63/64-127; causal diagonal
       blocks get an additive -1e5 mask via one extra [128,128] matmul
       per head into the score PSUM (keeps DVE out of the exp->PV
       chain); one Exp over the paired [128,1024] PSUM (split in two 2D
       activations when the diagonal narrows; 3D APs run at half rate);
       PV accumulates [V_h | ones]^T @ attnT into po_h[128,512] whose
       rows 64:128 hold the softmax denominator l; normalization is
       copy+reciprocal+multiply on DVE straight out of PSUM.
  a2a/proj: output resharded by token chunks — b0 in 2 half-T chunks,
       b1 per-qj (128KB each), with b1 computed in order qj1,qj2,qj3,
       qj0 so the LAST chunk is the cheapest attention block and its
       AllToAll (the only one on the critical tail) is small and hits a
       warm CC stream. Each chunk's local W_proj^T @ A + bias runs as
       filler under later attention. A tiny warmup AllToAll absorbs the
       first-collective/barrier latency.
Host reassembles the token chunks.
"""

import numpy as np
from collections import deque

B, T, C, H = 2, 2048, 1024, 16
D = C // H            # 64
NCORES = 8
HL = H // NCORES      # 2 heads per core
NT = B * T            # 4096
NQ = T // 512         # 4 q-blocks of 512 per b
NK = T // 128         # 16 k-chunks of 128 per b
SCALE = float(D) ** -0.5
MASKNEG = -1.0e5      # additive causal mask; exp(SCALE*(s+MASKNEG)) == 0

# output chunks: (b, [(token_lo, token_hi), ...], outT column offset)
# 3 data collectives: per-op latency (not size) dominates each AllToAll
# and the CC stream serializes ops, so few well-spaced collectives beat
# many small ones.  b1 computes qj1,qj2,qj3 first and qj0 (the cheapest
# causal block) last; the final chunk (qj3 + qj0) is staged in two
# parts and its AllToAll fires at the very end onto an idle stream,
# overlapped by the b1m projection.
CHUNKS = {
    "b0": (0, [(0, 2048)], 0),
    "b1m": (1, [(512, 1536)], 256),
    "b1e": (1, [(1536, 2048), (0, 512)], 384),
}

_cache = {}


def _build(mode: str):
    """mode: 'causal' | 'none' (all-ones mask)."""
    import concourse.mybir as mybir
    import concourse.tile as tile
    from concourse import bacc

    f32 = mybir.dt.float32
    mdt = mybir.dt.bfloat16

    nc = bacc.Bacc("TRN2", target_bir_lowering=False, debug=False,
                   num_devices=NCORES)
    # host-packed: xh[p, g, nl, kc, u] = x^T[kc*128+p, g*2048+nl*512+u]
    xh = nc.dram_tensor("xh", [128, B * 4 * 8 * 512], mdt,
                        kind="ExternalInput").ap()
    # host-permuted: wqkv[p, kc*384 + m] = W_qkv_slice[kc*128 + p, m]
    wqkv = nc.dram_tensor("wqkv", [128, 8 * 3 * HL * D], mdt,
                          kind="ExternalInput").ap()
    # host-permuted: wp[p, kc*1024 + o] = W_proj[kc*128 + p, o]
    wp = nc.dram_tensor("wp", [128, 8 * C], mdt,
                        kind="ExternalInput").ap()
    bias = nc.dram_tensor("bias", [128, NCORES], f32,
                          kind="ExternalInput").ap()
    # cols 0:128 identity; cols 128:256 additive causal mask
    cmask = nc.dram_tensor("cmask", [128, 256], mdt,
                           kind="ExternalInput").ap()
    outT = nc.dram_tensor("outT", [C, 512], f32,
                          kind="ExternalOutput").ap()

    causal = mode == "causal"
    Exp = mybir.ActivationFunctionType.Exp
    xh5 = xh.rearrange("p (g nl a u) -> p g nl a u", g=B, nl=4, a=8)

    with tile.TileContext(nc) as tc, \
         nc.allow_low_precision(reason="bf16 matmul path, tol 2e-2"):
        with tc.tile_pool(name="persist", bufs=1) as persist, \
             tc.tile_pool(name="dram", bufs=1, space="DRAM") as dram:
            q_sb = persist.tile([128, NT], mdt)
            k_sb = persist.tile([128, NT], mdt)
            # V^T tiles: vboth[p, h, j, 0:64] = V_h d-columns for k-chunk
            # j; vboth[p, h, j, 64:128] = ones (PV output rows 64:128
            # then hold the softmax denominator l).
            vboth = persist.tile([128, HL, B * NK, 128], mdt)
            cm_sb = persist.tile([128, 256], mdt)
            wqkv_sb = persist.tile([128, 8, 3 * HL * D], mdt)
            wp_sb = persist.tile([128, 8, C], mdt)
            bias_sb = persist.tile([128, NCORES], f32)
            a2a_ins = {}
            a2a_outs = {}
            for ck, (_, parts, _c) in CHUNKS.items():
                w = sum(hi - lo for lo, hi in parts) // NCORES
                a2a_ins[ck] = dram.tile([NCORES * 128, w], mdt,
                                        name=f"a2a_in_{ck}")
                a2a_outs[ck] = dram.tile([NCORES * 128, w], mdt,
                                         name=f"a2a_out_{ck}")
            warm_in = dram.tile([NCORES, 16], mdt)
            warm_out = dram.tile([NCORES, 16], mdt)

            # wqkv on the scalar HWDGE ring: parallel to the x loads on
            # the sync ring, faster than the gpsimd SWDGE path
            nc.scalar.dma_start(out=wqkv_sb[:],
                                in_=wqkv.rearrange("p (a n) -> p a n",
                                                   a=8))
            nc.gpsimd.dma_start(out=cm_sb[:], in_=cmask[:])
            nc.gpsimd.dma_start(out=bias_sb[:], in_=bias[:])
            nc.gpsimd.dma_start(out=wp_sb[:],
                                in_=wp.rearrange("p (a n) -> p a n", a=8))
            for h in range(HL):
                nc.vector.memset(vboth[:, h, :, 64:128], 1.0)
            ident = cm_sb[:, 0:128]
            maskadd = cm_sb[:, 128:256]

            # PSUM layout (8 banks):
            #   mm1 (2 banks): qkv chains + transposes + proj accum
            #   sc  (4 banks): paired score tiles [128,1024] x2 in flight
            #   po  (2 banks): po_h0 / po_h1 accumulators
            with tc.tile_pool(name="mm1", bufs=2, space="PSUM") as mm1, \
                 tc.tile_pool(name="sc_psum", bufs=2, space="PSUM") as scp, \
                 tc.tile_pool(name="po_psum", bufs=1, space="PSUM") as pop, \
                 tc.tile_pool(name="xn_pool", bufs=2) as xp, \
                 tc.tile_pool(name="vtmp_pool", bufs=2) as vpool, \
                 tc.tile_pool(name="at_pool", bufs=6) as apool, \
                 tc.tile_pool(name="rb_pool", bufs=2) as rbp, \
                 tc.tile_pool(name="a_pool", bufs=2) as ap_pool, \
                 tc.tile_pool(name="agt_pool", bufs=3) as agp, \
                 tc.tile_pool(name="out_pool", bufs=8) as outp:

                xns = {}

                def xn_dmas(g):
                    """Load x^T for token group g; 8KB contiguous lines
                    per partition.  nl0 is split in two so the first
                    qkv chains can start a little earlier."""
                    xn = xp.tile([128, 4, 8, 512], mdt, tag="xn",
                                 name="xn")
                    xns[g] = xn
                    if g == 0:
                        nc.sync.dma_start(out=xn[:, 0, 0:4, :],
                                          in_=xh5[:, g, 0, 0:4, :])
                        nc.sync.dma_start(out=xn[:, 0, 4:8, :],
                                          in_=xh5[:, g, 0, 4:8, :])
                    else:
                        nc.sync.dma_start(out=xn[:, 0, :, :],
                                          in_=xh5[:, g, 0, :, :])
                    for nl_ in range(1, 4):
                        nc.sync.dma_start(out=xn[:, nl_, :, :],
                                          in_=xh5[:, g, nl_, :, :])

                def qkv_chain(g, nl, m):
                    """One 8-matmul chain: q/k/v (m=0/1/2) for tokens
                    g*2048+nl*512 .. +512."""
                    xn = xns[g]
                    n = g * 4 + nl
                    tok = slice(n * 512, (n + 1) * 512)
                    ps = mm1.tile([128, 512], f32, tag="ps", name="ps")
                    for kc in range(8):
                        nc.tensor.matmul(
                            ps[:],
                            wqkv_sb[:, kc, m * 128:(m + 1) * 128],
                            xn[:, nl, kc, :],
                            start=(kc == 0), stop=(kc == 7))
                    if m == 0:
                        nc.vector.tensor_copy(q_sb[:, tok], ps[:])
                    elif m == 1:
                        nc.vector.tensor_copy(k_sb[:, tok], ps[:])
                    else:
                        vtmp = vpool.tile([128, 512], mdt, tag="vtmp",
                                          name="vtmp")
                        nc.vector.tensor_copy(vtmp[:], ps[:])
                        bb = n // NQ
                        for s in range(4):
                            j = bb * NK + (n % NQ) * 4 + s
                            pt = mm1.tile([128, 128], mdt, tag="ps",
                                          name="pt")
                            nc.tensor.transpose(
                                pt[:], vtmp[:, s * 128:(s + 1) * 128],
                                ident)
                            nc.vector.tensor_copy(
                                vboth[:, :, j, 0:64],
                                pt[:].rearrange("p (h d) -> p h d", h=2))

                def qkv_thunks(g, nl):
                    return [lambda m=m: qkv_chain(g, nl, m)
                            for m in range(3)]

                def attn_qj(b, qj, a_sb, filler=(), fill_from=0):
                    """Score/exp/PV loop + normalization for (b, qj).
                    Pops one filler thunk per ki iteration starting at
                    iteration fill_from; remaining thunks run after."""
                    filler = deque(filler)
                    last_ki = 4 * qj + 3 if causal else NK - 1
                    po0 = pop.tile([128, 512], f32, tag="po0", name="po0")
                    po1 = pop.tile([128, 512], f32, tag="po1", name="po1")
                    pos = [po0, po1]

                    def emit_pv(ki, at, st):
                        vj = b * NK + ki
                        for h in range(2):
                            nc.tensor.matmul(
                                pos[h][:, st:512],
                                vboth[:, h, vj, :],
                                at[:, 512 * h + st:512 * h + 512],
                                start=(ki == 0), stop=(ki == last_ki))

                    # software-pipelined by one stage: scores(ki+1) sit
                    # ahead of PV(ki) in the PE FIFO so the PE never
                    # stalls on exp(ki)
                    pend = None
                    for ki in range(last_ki + 1):
                        diag = causal and ki >= 4 * qj
                        st = (ki - 4 * qj) * 128 if diag else 0
                        kc_ = slice(b * T + ki * 128,
                                    b * T + (ki + 1) * 128)
                        qc = slice(b * T + qj * 512 + st,
                                   b * T + (qj + 1) * 512)
                        sc = scp.tile([128, 1024], f32, tag="sc",
                                      name="sc")
                        nc.tensor.matmul(
                            sc[:, st:512], k_sb[0:64, kc_],
                            q_sb[0:64, qc], start=True, stop=not diag)
                        nc.tensor.matmul(
                            sc[:, 512 + st:1024], k_sb[64:128, kc_],
                            q_sb[64:128, qc], start=True, stop=not diag)
                        if diag:
                            nc.tensor.matmul(
                                sc[:, st:st + 128], maskadd, ident,
                                start=False, stop=True)
                            nc.tensor.matmul(
                                sc[:, 512 + st:512 + st + 128], maskadd,
                                ident, start=False, stop=True)
                        if pend is not None:
                            emit_pv(*pend)
                        if filler and ki >= fill_from:
                            filler.popleft()()
                        at = apool.tile([128, 1024], mdt, tag="at",
                                        name="at")
                        if st == 0:
                            nc.scalar.activation(at[:], sc[:], Exp,
                                                 scale=SCALE)
                        elif st <= 256:
                            for h in range(2):
                                nc.scalar.activation(
                                    at[:, 512 * h + st:512 * h + 512],
                                    sc[:, 512 * h + st:512 * h + 512],
                                    Exp, scale=SCALE)
                        else:
                            at3 = at[:].rearrange(
                                "p (c t) -> p c t", c=2)[:, :, st:512]
                            sc3 = sc[:].rearrange(
                                "p (c t) -> p c t", c=2)[:, :, st:512]
                            nc.scalar.activation(at3, sc3, Exp,
                                                 scale=SCALE)
                        pend = (ki, at, st)
                    emit_pv(*pend)
                    # normalize: po rows 64:128 are l replicated
                    for h in range(2):
                        lrep = rbp.tile([64, 512], f32, tag="lrep",
                                        name="lrep")
                        nc.vector.tensor_copy(lrep[:], pos[h][64:128, :])
                        rb = rbp.tile([64, 512], f32, tag="rb", name="rb")
                        nc.vector.reciprocal_approx_fast(rb[:], lrep[:])
                        nc.vector.tensor_mul(
                            a_sb[64 * h:64 * h + 64,
                                 qj * 512:(qj + 1) * 512],
                            pos[h][0:64, :], rb[:])
                    while filler:
                        filler.popleft()()

                def stage_part(ck, pi, a_sb):
                    _, parts, _c = CHUNKS[ck]
                    off = sum(h - l for l, h in parts[:pi]) // NCORES
                    lo, hi = parts[pi]
                    pw = (hi - lo) // NCORES
                    inv = a2a_ins[ck][:].rearrange("(c p) t -> p c t",
                                                   p=128)
                    nc.sync.dma_start(
                        out=inv[:, :, off:off + pw],
                        in_=a_sb[:, lo:hi].rearrange("p (c t) -> p c t",
                                                     c=NCORES))

                def a2a(ck):
                    nc.gpsimd.collective_compute(
                        "AllToAll", mybir.AluOpType.bypass,
                        replica_groups=[list(range(NCORES))],
                        ins=[a2a_ins[ck].opt()], outs=[a2a_outs[ck].opt()])

                agts = {}

                def agt_load(ck):
                    _, parts, _c = CHUNKS[ck]
                    w = sum(hi - lo for lo, hi in parts) // NCORES
                    agt = agp.tile([128, 8, w], mdt, tag="agt",
                                   name="agt")
                    agts[ck] = agt
                    nc.gpsimd.dma_start(
                        out=agt[:],
                        in_=a2a_outs[ck][:].rearrange("(c p) t -> p c t",
                                                      p=128))

                def proj_o(ck, o):
                    """One output tile of the local projection for this
                    core's token chunk ck."""
                    _, parts, col = CHUNKS[ck]
                    w = sum(hi - lo for lo, hi in parts) // NCORES
                    agt = agts[ck]
                    pr = mm1.tile([128, w], f32, tag="ps", name="pr")
                    for kc in range(8):
                        nc.tensor.matmul(
                            pr[:], wp_sb[:, kc, o * 128:(o + 1) * 128],
                            agt[:, kc, :],
                            start=(kc == 0), stop=(kc == 7))
                    ot = outp.tile([128, w], f32, tag="ot", name="ot")
                    nc.vector.tensor_scalar_add(ot[:], pr[:],
                                                bias_sb[:, o:o + 1])
                    nc.sync.dma_start(
                        out=outT[o * 128:(o + 1) * 128, col:col + w],
                        in_=ot[:])

                def proj_thunks(ck):
                    return [lambda o=o: proj_o(ck, o) for o in range(8)]

                # tiny warmup collective to absorb first-op CC latency
                nc.gpsimd.collective_compute(
                    "AllToAll", mybir.AluOpType.bypass,
                    replica_groups=[list(range(NCORES))],
                    ins=[warm_in.opt()], outs=[warm_out.opt()])
                a_sb0 = ap_pool.tile([128, T], mdt, tag="a_sb",
                                     name="a_sb0")
                a_sb1 = ap_pool.tile([128, T], mdt, tag="a_sb",
                                     name="a_sb1")
                xn_dmas(0)
                for m in range(3):
                    qkv_chain(0, 0, m)
                attn_qj(0, 0, a_sb0)
                for m in range(3):
                    qkv_chain(0, 1, m)
                attn_qj(0, 1, a_sb0, qkv_thunks(0, 2))
                xn_dmas(1)
                attn_qj(0, 2, a_sb0, qkv_thunks(0, 3) + qkv_thunks(1, 0))
                attn_qj(0, 3, a_sb0, qkv_thunks(1, 1) + qkv_thunks(1, 2))
                stage_part("b0", 0, a_sb0)
                a2a("b0")
                agt_load("b0")
                # b1 in order qj1, qj2, qj3, qj0 — cheapest block last
                attn_qj(1, 1, a_sb1, qkv_thunks(1, 3))
                attn_qj(1, 2, a_sb1)
                stage_part("b1m", 0, a_sb1)
                a2a("b1m")
                agt_load("b1m")
                attn_qj(1, 3, a_sb1, proj_thunks("b0"), fill_from=6)
                stage_part("b1e", 0, a_sb1)
                attn_qj(1, 0, a_sb1)
                stage_part("b1e", 1, a_sb1)
                a2a("b1e")
                # proj(b1m) compute hides the tail AllToAll's flight
                for o in range(8):
                    proj_o("b1m", o)
                agt_load("b1e")
                for o in range(8):
                    proj_o("b1e", o)

    nc.compile()
    return nc


def _get_program(mode: str):
    if mode not in _cache:
        _cache[mode] = _build(mode)
    return _cache[mode]


def kernel(**inputs):
    import ml_dtypes
    from concourse.bass_utils import run_bass_kernel_spmd

    bf16 = ml_dtypes.bfloat16

    x = np.asarray(inputs["x"], dtype=np.float32)
    mask = np.asarray(inputs["causal_mask"])
    Wqkv = np.asarray(inputs["W_qkv"], dtype=np.float32)
    Wp = np.asarray(inputs["W_proj"], dtype=np.float32)
    bp = np.asarray(inputs["b_proj"], dtype=np.float32)

    m2 = mask.reshape(T, T)
    if np.all(m2 != 0):
        mode = "none"
    else:
        tril = np.tril(np.ones((T, T), dtype=m2.dtype))
        if np.array_equal(m2, tril):
            mode = "causal"
        else:
            raise NotImplementedError("general mask not supported")

    nc = _get_program(mode)

    # xh[p, g, nl, a, u] = x^T[a*128+p, g*2048+nl*512+u]
    xT = np.ascontiguousarray(x.reshape(NT, C).T)           # [1024, 4096]
    xh = np.ascontiguousarray(
        xT.reshape(8, 128, B, 4, 512).transpose(1, 2, 3, 0, 4)
        .reshape(128, B * 4 * 8 * 512)).astype(bf16)

    # [128,256]: identity | additive causal mask (-1e5 above diagonal)
    p = np.arange(128)[:, None]
    f = np.arange(128)[None, :]
    madd = np.where(p < f, MASKNEG, 0.0).astype(np.float32)
    cm = np.concatenate(
        [np.eye(128, dtype=np.float32), madd], axis=1).astype(bf16)

    Wq = Wqkv[:, 0 * C:1 * C]
    Wk = Wqkv[:, 1 * C:2 * C]
    Wv = Wqkv[:, 2 * C:3 * C]
    wp_bf = np.ascontiguousarray(
        Wp.reshape(8, 128, C).transpose(1, 0, 2).reshape(128, 8 * C)
    ).astype(bf16)
    bias_h = np.ascontiguousarray(bp.reshape(NCORES, 128).T)

    in_maps = []
    for i in range(NCORES):
        hcols = slice(2 * i * D, (2 * i + 2) * D)  # this core's 2 heads
        wqkv_i = np.concatenate(
            [Wq[:, hcols], Wk[:, hcols], Wv[:, hcols]], axis=1)
        wqkv_p = np.ascontiguousarray(
            wqkv_i.reshape(8, 128, 3 * HL * D).transpose(1, 0, 2)
            .reshape(128, 8 * 3 * HL * D)).astype(bf16)
        in_maps.append({
            "xh": xh,
            "wqkv": wqkv_p,
            "wp": wp_bf,
            "bias": bias_h,
            "cmask": cm,
        })

    res = run_bass_kernel_spmd(nc, in_maps, list(range(NCORES)))

    out = np.empty((B, T, C), dtype=np.float32)
    for i in range(NCORES):
        oT = res.results[i]["outT"]  # [C, 512] f32
        for _ck, (b, parts, col) in CHUNKS.items():
            c = col
            for lo, hi in parts:
                w = (hi - lo) // NCORES
                out[b, lo + i * w:lo + (i + 1) * w, :] = \
                    oT[:, c:c + w].T
                c += w
    return out


# revision 25
# speedup vs baseline: 1.0473x; 1.0473x over previous
"""Multi-head causal self-attention on 8 Trainium2 NeuronCores.

Tensor-parallel over heads: core i owns heads (2i, 2i+1). bf16 matmul
operands throughout (fp32 PSUM accumulation); harness tolerance 2e-2.

Per core (v5 — deep-pipelined emission with filler thunks):
  One merged pipeline: attention for batch 0 starts right after the
  first 512-token QKV block, so the Exp activation engine (the
  attention bottleneck at ~1.1us per ki step) ramps early.  Remaining
  QKV chains and the projection matmuls are emitted as small "filler"
  thunks popped between ki iterations of later attention blocks — the
  per-engine queues are strict FIFO, so placement order is execution
  order, and every filler is placed late enough that its dependencies
  (DMAs, collectives) are already satisfied when it reaches the queue
  head.

  qkv: per 512-token block, q/k/v = (W_slice^T @ x^T); x is host-packed
       [128, g, nl, kc, 512] so each load is one DMA with 8KB
       contiguous lines (descriptor-count, not bytes, dominates
       dispatch cost). vT via PE transposes.
  attn: per (b, qj, ki): scoresT for both heads as two row-tiled K=64
       matmuls concurrent on PE rows 0-63/64-127; causal diagonal
       blocks get an additive -1e5 mask via one extra [128,128] matmul
       per head into the score PSUM (keeps DVE out of the exp->PV
       chain); one Exp over the paired [128,1024] PSUM (split in two 2D
       activations when the diagonal narrows; 3D APs run at half rate);
       PV accumulates [V_h | ones]^T @ attnT into po_h[128,512] whose
       rows 64:128 hold the softmax denominator l; normalization is
       copy+reciprocal+multiply on DVE straight out of PSUM.
  a2a/proj: output resharded by token chunks — b0 in 2 half-T chunks,
       b1 per-qj (128KB each), with b1 computed in order qj1,qj2,qj3,
       qj0 so the LAST chunk is the cheapest attention block and its
       AllToAll (the only one on the critical tail) is small and hits a
       warm CC stream. Each chunk's local W_proj^T @ A + bias runs as
       filler under later attention. A tiny warmup AllToAll absorbs the
       first-collective/barrier latency.
Host reassembles the token chunks.
"""

import numpy as np
from collections import deque

B, T, C, H = 2, 2048, 1024, 16
D = C // H            # 64
NCORES = 8
HL = H // NCORES      # 2 heads per core
NT = B * T            # 4096
NQ = T // 512         # 4 q-blocks of 512 per b
NK = T // 128         # 16 k-chunks of 128 per b
SCALE = float(D) ** -0.5
MASKNEG = -1.0e5      # additive causal mask; exp(SCALE*(s+MASKNEG)) == 0

# output chunks: (b, [(token_lo, token_hi), ...], outT column offset)
# 3 data collectives: per-op latency (not size) dominates each AllToAll
# and the CC stream serializes ops, so few well-spaced collectives beat
# many small ones.  b1 computes qj1,qj2,qj3 first and qj0 (the cheapest
# causal block) last; the final chunk (qj3 + qj0) is staged in two
# parts and its AllToAll fires at the very end onto an idle stream,
# overlapped by the b1m projection.
CHUNKS = {
    "b0h0": (0, [(0, 1024)], 0),
    "b0h1": (0, [(1024, 2048)], 128),
    "b1q1": (1, [(512, 1024)], 256),
    "b1m": (1, [(1024, 2048)], 320),
    "b1q0": (1, [(0, 512)], 448),
}

_cache = {}


def _build(mode: str):
    """mode: 'causal' | 'none' (all-ones mask)."""
    import concourse.mybir as mybir
    import concourse.tile as tile
    from concourse import bacc

    f32 = mybir.dt.float32
    mdt = mybir.dt.bfloat16

    nc = bacc.Bacc("TRN2", target_bir_lowering=False, debug=False,
                   num_devices=NCORES)
    # host-packed: xh[p, g, nl, kc, u] = x^T[kc*128+p, g*2048+nl*512+u]
    xh = nc.dram_tensor("xh", [128, B * 4 * 8 * 512], mdt,
                        kind="ExternalInput").ap()
    # host-permuted: wqkv[p, kc*384 + m] = W_qkv_slice[kc*128 + p, m]
    wqkv = nc.dram_tensor("wqkv", [128, 8 * 3 * HL * D], mdt,
                          kind="ExternalInput").ap()
    # host-permuted: wp[p, kc*1024 + o] = W_proj[kc*128 + p, o]
    wp = nc.dram_tensor("wp", [128, 8 * C], mdt,
                        kind="ExternalInput").ap()
    bias = nc.dram_tensor("bias", [128, NCORES], f32,
                          kind="ExternalInput").ap()
    # cols 0:128 identity; cols 128:256 additive causal mask
    cmask = nc.dram_tensor("cmask", [128, 256], mdt,
                           kind="ExternalInput").ap()
    outT = nc.dram_tensor("outT", [C, 512], f32,
                          kind="ExternalOutput").ap()

    causal = mode == "causal"
    Exp = mybir.ActivationFunctionType.Exp
    xh5 = xh.rearrange("p (g nl a u) -> p g nl a u", g=B, nl=4, a=8)

    with tile.TileContext(nc) as tc, \
         nc.allow_low_precision(reason="bf16 matmul path, tol 2e-2"):
        with tc.tile_pool(name="persist", bufs=1) as persist, \
             tc.tile_pool(name="dram", bufs=1, space="DRAM") as dram:
            q_sb = persist.tile([128, NT], mdt)
            k_sb = persist.tile([128, NT], mdt)
            # V^T tiles: vboth[p, h, j, 0:64] = V_h d-columns for k-chunk
            # j; vboth[p, h, j, 64:128] = ones (PV output rows 64:128
            # then hold the softmax denominator l).
            vboth = persist.tile([128, HL, B * NK, 128], mdt)
            cm_sb = persist.tile([128, 256], mdt)
            wqkv_sb = persist.tile([128, 8, 3 * HL * D], mdt)
            wp_sb = persist.tile([128, 8, C], mdt)
            bias_sb = persist.tile([128, NCORES], f32)
            a2a_ins = {}
            a2a_outs = {}
            for ck, (_, parts, _c) in CHUNKS.items():
                w = sum(hi - lo for lo, hi in parts) // NCORES
                a2a_ins[ck] = dram.tile([NCORES * 128, w], mdt,
                                        name=f"a2a_in_{ck}")
                a2a_outs[ck] = dram.tile([NCORES * 128, w], mdt,
                                         name=f"a2a_out_{ck}")
            warm_in = dram.tile([NCORES, 16], mdt)
            warm_out = dram.tile([NCORES, 16], mdt)

            # wqkv on the scalar HWDGE ring: parallel to the x loads on
            # the sync ring, faster than the gpsimd SWDGE path
            nc.scalar.dma_start(out=wqkv_sb[:],
                                in_=wqkv.rearrange("p (a n) -> p a n",
                                                   a=8))
            nc.gpsimd.dma_start(out=cm_sb[:], in_=cmask[:])
            nc.gpsimd.dma_start(out=bias_sb[:], in_=bias[:])
            nc.gpsimd.dma_start(out=wp_sb[:],
                                in_=wp.rearrange("p (a n) -> p a n", a=8))
            for h in range(HL):
                nc.vector.memset(vboth[:, h, :, 64:128], 1.0)
            ident = cm_sb[:, 0:128]
            maskadd = cm_sb[:, 128:256]

            # PSUM layout (8 banks):
            #   mm1 (2 banks): qkv chains + transposes + proj accum
            #   sc  (4 banks): paired score tiles [128,1024] x2 in flight
            #   po  (2 banks): po_h0 / po_h1 accumulators
            with tc.tile_pool(name="mm1", bufs=2, space="PSUM") as mm1, \
                 tc.tile_pool(name="sc_psum", bufs=2, space="PSUM") as scp, \
                 tc.tile_pool(name="po_psum", bufs=1, space="PSUM") as pop, \
                 tc.tile_pool(name="xn_pool", bufs=2) as xp, \
                 tc.tile_pool(name="vtmp_pool", bufs=2) as vpool, \
                 tc.tile_pool(name="at_pool", bufs=6) as apool, \
                 tc.tile_pool(name="rb_pool", bufs=2) as rbp, \
                 tc.tile_pool(name="a_pool", bufs=2) as ap_pool, \
                 tc.tile_pool(name="agt_pool", bufs=3) as agp, \
                 tc.tile_pool(name="out_pool", bufs=8) as outp:

                xns = {}

                def xn_dmas(g):
                    """Load x^T for token group g; 8KB contiguous lines
                    per partition.  nl0 is split in two so the first
                    qkv chains can start a little earlier."""
                    xn = xp.tile([128, 4, 8, 512], mdt, tag="xn",
                                 name="xn")
                    xns[g] = xn
                    if g == 0:
                        nc.sync.dma_start(out=xn[:, 0, 0:4, :],
                                          in_=xh5[:, g, 0, 0:4, :])
                        nc.sync.dma_start(out=xn[:, 0, 4:8, :],
                                          in_=xh5[:, g, 0, 4:8, :])
                    else:
                        nc.sync.dma_start(out=xn[:, 0, :, :],
                                          in_=xh5[:, g, 0, :, :])
                    for nl_ in range(1, 4):
                        nc.sync.dma_start(out=xn[:, nl_, :, :],
                                          in_=xh5[:, g, nl_, :, :])

                def qkv_chain(g, nl, m):
                    """One 8-matmul chain: q/k/v (m=0/1/2) for tokens
                    g*2048+nl*512 .. +512."""
                    xn = xns[g]
                    n = g * 4 + nl
                    tok = slice(n * 512, (n + 1) * 512)
                    ps = mm1.tile([128, 512], f32, tag="ps", name="ps")
                    for kc in range(8):
                        nc.tensor.matmul(
                            ps[:],
                            wqkv_sb[:, kc, m * 128:(m + 1) * 128],
                            xn[:, nl, kc, :],
                            start=(kc == 0), stop=(kc == 7))
                    if m == 0:
                        nc.vector.tensor_copy(q_sb[:, tok], ps[:])
                    elif m == 1:
                        nc.vector.tensor_copy(k_sb[:, tok], ps[:])
                    else:
                        vtmp = vpool.tile([128, 512], mdt, tag="vtmp",
                                          name="vtmp")
                        nc.vector.tensor_copy(vtmp[:], ps[:])
                        bb = n // NQ
                        for s in range(4):
                            j = bb * NK + (n % NQ) * 4 + s
                            pt = mm1.tile([128, 128], mdt, tag="ps",
                                          name="pt")
                            nc.tensor.transpose(
                                pt[:], vtmp[:, s * 128:(s + 1) * 128],
                                ident)
                            nc.vector.tensor_copy(
                                vboth[:, :, j, 0:64],
                                pt[:].rearrange("p (h d) -> p h d", h=2))

                def qkv_thunks(g, nl):
                    return [lambda m=m: qkv_chain(g, nl, m)
                            for m in range(3)]

                def attn_qj(b, qj, a_sb, filler=(), fill_from=0):
                    """Score/exp/PV loop + normalization for (b, qj).
                    Pops one filler thunk per ki iteration starting at
                    iteration fill_from; remaining thunks run after."""
                    filler = deque(filler)
                    last_ki = 4 * qj + 3 if causal else NK - 1
                    po0 = pop.tile([128, 512], f32, tag="po0", name="po0")
                    po1 = pop.tile([128, 512], f32, tag="po1", name="po1")
                    pos = [po0, po1]

                    def emit_pv(ki, at, st):
                        vj = b * NK + ki
                        for h in range(2):
                            nc.tensor.matmul(
                                pos[h][:, st:512],
                                vboth[:, h, vj, :],
                                at[:, 512 * h + st:512 * h + 512],
                                start=(ki == 0), stop=(ki == last_ki))

                    # software-pipelined by one stage: scores(ki+1) sit
                    # ahead of PV(ki) in the PE FIFO so the PE never
                    # stalls on exp(ki)
                    pend = None
                    for ki in range(last_ki + 1):
                        diag = causal and ki >= 4 * qj
                        st = (ki - 4 * qj) * 128 if diag else 0
                        kc_ = slice(b * T + ki * 128,
                                    b * T + (ki + 1) * 128)
                        qc = slice(b * T + qj * 512 + st,
                                   b * T + (qj + 1) * 512)
                        sc = scp.tile([128, 1024], f32, tag="sc",
                                      name="sc")
                        nc.tensor.matmul(
                            sc[:, st:512], k_sb[0:64, kc_],
                            q_sb[0:64, qc], start=True, stop=not diag)
                        nc.tensor.matmul(
                            sc[:, 512 + st:1024], k_sb[64:128, kc_],
                            q_sb[64:128, qc], start=True, stop=not diag)
                        if diag:
                            nc.tensor.matmul(
                                sc[:, st:st + 128], maskadd, ident,
                                start=False, stop=True)
                            nc.tensor.matmul(
                                sc[:, 512 + st:512 + st + 128], maskadd,
                                ident, start=False, stop=True)
                        if pend is not None:
                            emit_pv(*pend)
                        if filler and ki >= fill_from:
                            filler.popleft()()
                        at = apool.tile([128, 1024], mdt, tag="at",
                                        name="at")
                        if st == 0:
                            nc.scalar.activation(at[:], sc[:], Exp,
                                                 scale=SCALE)
                        elif st <= 256:
                            for h in range(2):
                                nc.scalar.activation(
                                    at[:, 512 * h + st:512 * h + 512],
                                    sc[:, 512 * h + st:512 * h + 512],
                                    Exp, scale=SCALE)
                        else:
                            at3 = at[:].rearrange(
                                "p (c t) -> p c t", c=2)[:, :, st:512]
                            sc3 = sc[:].rearrange(
                                "p (c t) -> p c t", c=2)[:, :, st:512]
                            nc.scalar.activation(at3, sc3, Exp,
                                                 scale=SCALE)
                        pend = (ki, at, st)
                    emit_pv(*pend)
                    # normalize: po rows 64:128 are l replicated
                    for h in range(2):
                        lrep = rbp.tile([64, 512], f32, tag="lrep",
                                        name="lrep")
                        nc.vector.tensor_copy(lrep[:], pos[h][64:128, :])
                        rb = rbp.tile([64, 512], f32, tag="rb", name="rb")
                        nc.vector.reciprocal_approx_fast(rb[:], lrep[:])
                        nc.vector.tensor_mul(
                            a_sb[64 * h:64 * h + 64,
                                 qj * 512:(qj + 1) * 512],
                            pos[h][0:64, :], rb[:])
                    while filler:
                        filler.popleft()()

                def stage_part(ck, pi, a_sb):
                    _, parts, _c = CHUNKS[ck]
                    off = sum(h - l for l, h in parts[:pi]) // NCORES
                    lo, hi = parts[pi]
                    pw = (hi - lo) // NCORES
                    inv = a2a_ins[ck][:].rearrange("(c p) t -> p c t",
                                                   p=128)
                    nc.sync.dma_start(
                        out=inv[:, :, off:off + pw],
                        in_=a_sb[:, lo:hi].rearrange("p (c t) -> p c t",
                                                     c=NCORES))

                def a2a(ck):
                    nc.gpsimd.collective_compute(
                        "AllToAll", mybir.AluOpType.bypass,
                        replica_groups=[list(range(NCORES))],
                        ins=[a2a_ins[ck].opt()], outs=[a2a_outs[ck].opt()])

                agts = {}

                def agt_load(ck):
                    _, parts, _c = CHUNKS[ck]
                    w = sum(hi - lo for lo, hi in parts) // NCORES
                    agt = agp.tile([128, 8, w], mdt, tag="agt",
                                   name="agt")
                    agts[ck] = agt
                    nc.gpsimd.dma_start(
                        out=agt[:],
                        in_=a2a_outs[ck][:].rearrange("(c p) t -> p c t",
                                                      p=128))

                def proj_o(ck, o):
                    """One output tile of the local projection for this
                    core's token chunk ck."""
                    _, parts, col = CHUNKS[ck]
                    w = sum(hi - lo for lo, hi in parts) // NCORES
                    agt = agts[ck]
                    pr = mm1.tile([128, w], f32, tag="ps", name="pr")
                    for kc in range(8):
                        nc.tensor.matmul(
                            pr[:], wp_sb[:, kc, o * 128:(o + 1) * 128],
                            agt[:, kc, :],
                            start=(kc == 0), stop=(kc == 7))
                    ot = outp.tile([128, w], f32, tag="ot", name="ot")
                    nc.vector.tensor_scalar_add(ot[:], pr[:],
                                                bias_sb[:, o:o + 1])
                    nc.sync.dma_start(
                        out=outT[o * 128:(o + 1) * 128, col:col + w],
                        in_=ot[:])

                def proj_thunks(ck):
                    return [lambda o=o: proj_o(ck, o) for o in range(8)]

                # tiny warmup collective to absorb first-op CC latency
                nc.gpsimd.collective_compute(
                    "AllToAll", mybir.AluOpType.bypass,
                    replica_groups=[list(range(NCORES))],
                    ins=[warm_in.opt()], outs=[warm_out.opt()])
                a_sb0 = ap_pool.tile([128, T], mdt, tag="a_sb",
                                     name="a_sb0")
                a_sb1 = ap_pool.tile([128, T], mdt, tag="a_sb",
                                     name="a_sb1")
                xn_dmas(0)
                for m in range(3):
                    qkv_chain(0, 0, m)
                attn_qj(0, 0, a_sb0)
                for m in range(3):
                    qkv_chain(0, 1, m)
                attn_qj(0, 1, a_sb0, qkv_thunks(0, 2))
                stage_part("b0h0", 0, a_sb0)
                a2a("b0h0")
                xn_dmas(1)
                attn_qj(0, 2, a_sb0, qkv_thunks(0, 3) + qkv_thunks(1, 0))
                attn_qj(0, 3, a_sb0, qkv_thunks(1, 1) + qkv_thunks(1, 2))
                stage_part("b0h1", 0, a_sb0)
                a2a("b0h1")
                agt_load("b0h0")
                # b1 in order qj1, qj2, qj3, qj0 — cheapest block last so
                # the tail AllToAll is small and the CC stream is drained
                attn_qj(1, 1, a_sb1, qkv_thunks(1, 3))
                stage_part("b1q1", 0, a_sb1)
                a2a("b1q1")
                agt_load("b0h1")
                attn_qj(1, 2, a_sb1, proj_thunks("b0h0"), fill_from=4)
                agt_load("b1q1")
                attn_qj(1, 3, a_sb1,
                        proj_thunks("b0h1") + proj_thunks("b1q1"),
                        fill_from=4)
                stage_part("b1m", 0, a_sb1)
                a2a("b1m")
                agt_load("b1m")
                attn_qj(1, 0, a_sb1)
                stage_part("b1q0", 0, a_sb1)
                a2a("b1q0")
                # proj(b1m) compute hides the tail AllToAll's flight
                for o in range(8):
                    proj_o("b1m", o)
                agt_load("b1q0")
                for o in range(8):
                    proj_o("b1q0", o)

    nc.compile()
    return nc


def _get_program(mode: str):
    if mode not in _cache:
        _cache[mode] = _build(mode)
    return _cache[mode]


def kernel(**inputs):
    import ml_dtypes
    from concourse.bass_utils import run_bass_kernel_spmd

    bf16 = ml_dtypes.bfloat16

    x = np.asarray(inputs["x"], dtype=np.float32)
    mask = np.asarray(inputs["causal_mask"])
    Wqkv = np.asarray(inputs["W_qkv"], dtype=np.float32)
    Wp = np.asarray(inputs["W_proj"], dtype=np.float32)
    bp = np.asarray(inputs["b_proj"], dtype=np.float32)

    m2 = mask.reshape(T, T)
    if np.all(m2 != 0):
        mode = "none"
    else:
        tril = np.tril(np.ones((T, T), dtype=m2.dtype))
        if np.array_equal(m2, tril):
            mode = "causal"
        else:
            raise NotImplementedError("general mask not supported")

    nc = _get_program(mode)

    # xh[p, g, nl, a, u] = x^T[a*128+p, g*2048+nl*512+u]
    xT = np.ascontiguousarray(x.reshape(NT, C).T)           # [1024, 4096]
    xh = np.ascontiguousarray(
        xT.reshape(8, 128, B, 4, 512).transpose(1, 2, 3, 0, 4)
        .reshape(128, B * 4 * 8 * 512)).astype(bf16)

    # [128,256]: identity | additive causal mask (-1e5 above diagonal)
    p = np.arange(128)[:, None]
    f = np.arange(128)[None, :]
    madd = np.where(p < f, MASKNEG, 0.0).astype(np.float32)
    cm = np.concatenate(
        [np.eye(128, dtype=np.float32), madd], axis=1).astype(bf16)

    Wq = Wqkv[:, 0 * C:1 * C]
    Wk = Wqkv[:, 1 * C:2 * C]
    Wv = Wqkv[:, 2 * C:3 * C]
    wp_bf = np.ascontiguousarray(
        Wp.reshape(8, 128, C).transpose(1, 0, 2).reshape(128, 8 * C)
    ).astype(bf16)
    bias_h = np.ascontiguousarray(bp.reshape(NCORES, 128).T)

    in_maps = []
    for i in range(NCORES):
        hcols = slice(2 * i * D, (2 * i + 2) * D)  # this core's 2 heads
        wqkv_i = np.concatenate(
            [Wq[:, hcols], Wk[:, hcols], Wv[:, hcols]], axis=1)
        wqkv_p = np.ascontiguousarray(
            wqkv_i.reshape(8, 128, 3 * HL * D).transpose(1, 0, 2)
            .reshape(128, 8 * 3 * HL * D)).astype(bf16)
        in_maps.append({
            "xh": xh,
            "wqkv": wqkv_p,
            "wp": wp_bf,
            "bias": bias_h,
            "cmask": cm,
        })

    res = run_bass_kernel_spmd(nc, in_maps, list(range(NCORES)))

    out = np.empty((B, T, C), dtype=np.float32)
    for i in range(NCORES):
        oT = res.results[i]["outT"]  # [C, 512] f32
        for _ck, (b, parts, col) in CHUNKS.items():
            c = col
            for lo, hi in parts:
                w = (hi - lo) // NCORES
                out[b, lo + i * w:lo + (i + 1) * w, :] = \
                    oT[:, c:c + w].T
                c += w
    return out


# revision 28
# speedup vs baseline: 1.0913x; 1.0420x over previous
"""Multi-head causal self-attention on 8 Trainium2 NeuronCores.

Tensor-parallel over heads: core i owns heads (2i, 2i+1). bf16 matmul
operands throughout (fp32 PSUM accumulation); harness tolerance 2e-2.

Per core (v5 — deep-pipelined emission with filler thunks):
  One merged pipeline: attention for batch 0 starts right after the
  first 512-token QKV block, so the Exp activation engine (the
  attention bottleneck at ~1.1us per ki step) ramps early.  Remaining
  QKV chains and the projection matmuls are emitted as small "filler"
  thunks popped between ki iterations of later attention blocks — the
  per-engine queues are strict FIFO, so placement order is execution
  order, and every filler is placed late enough that its dependencies
  (DMAs, collectives) are already satisfied when it reaches the queue
  head.

  qkv: per 512-token block, q/k/v = (W_slice^T @ x^T); x is host-packed
       [128, g, nl, kc, 512] so each load is one DMA with 8KB
       contiguous lines (descriptor-count, not bytes, dominates
       dispatch cost). vT via PE transposes.
  attn: per (b, qj, ki): scoresT for both heads as two row-tiled K=64
       matmuls concurrent on PE rows 0-63/64-127; causal diagonal
       blocks get an additive -1e5 mask via one extra [128,128] matmul
       per head into the score PSUM (keeps DVE out of the exp->PV
       chain); one Exp over the paired [128,1024] PSUM (split in two 2D
       activations when the diagonal narrows; 3D APs run at half rate);
       PV accumulates [V_h | ones]^T @ attnT into po_h[128,512] whose
       rows 64:128 hold the softmax denominator l; normalization is
       copy+reciprocal+multiply on DVE straight out of PSUM.
  a2a/proj: output resharded by token chunks — b0 in 2 half-T chunks,
       b1 per-qj (128KB each), with b1 computed in order qj1,qj2,qj3,
       qj0 so the LAST chunk is the cheapest attention block and its
       AllToAll (the only one on the critical tail) is small and hits a
       warm CC stream. Each chunk's local W_proj^T @ A + bias runs as
       filler under later attention. A tiny warmup AllToAll absorbs the
       first-collective/barrier latency.
Host reassembles the token chunks.
"""

import numpy as np
from collections import deque

B, T, C, H = 2, 2048, 1024, 16
D = C // H            # 64
NCORES = 8
HL = H // NCORES      # 2 heads per core
NT = B * T            # 4096
NQ = T // 512         # 4 q-blocks of 512 per b
NK = T // 128         # 16 k-chunks of 128 per b
SCALE = float(D) ** -0.5
MASKNEG = -1.0e5      # additive causal mask; exp(SCALE*(s+MASKNEG)) == 0

# output chunks: (b, [(token_lo, token_hi), ...], outT column offset)
# 3 data collectives: per-op latency (not size) dominates each AllToAll
# and the CC stream serializes ops, so few well-spaced collectives beat
# many small ones.  b1 computes qj1,qj2,qj3 first and qj0 (the cheapest
# causal block) last; the final chunk (qj3 + qj0) is staged in two
# parts and its AllToAll fires at the very end onto an idle stream,
# overlapped by the b1m projection.
CHUNKS = {
    "b0h0": (0, [(0, 1024)], 0),
    "b0h1": (0, [(1024, 2048)], 128),
    "b1q1": (1, [(512, 1024)], 256),
    "b1q2": (1, [(1024, 1536)], 320),
    "b1q3": (1, [(1536, 2048)], 384),
    "b1q0": (1, [(0, 512)], 448),
}

_cache = {}


def _build(mode: str):
    """mode: 'causal' | 'none' (all-ones mask)."""
    import concourse.mybir as mybir
    import concourse.tile as tile
    from concourse import bacc

    f32 = mybir.dt.float32
    mdt = mybir.dt.bfloat16

    nc = bacc.Bacc("TRN2", target_bir_lowering=False, debug=False,
                   num_devices=NCORES)
    # host-packed: xh[p, g, nl, kc, u] = x^T[kc*128+p, g*2048+nl*512+u]
    xh = nc.dram_tensor("xh", [128, B * 4 * 8 * 512], mdt,
                        kind="ExternalInput").ap()
    # host-permuted: wqkv[p, kc*384 + m] = W_qkv_slice[kc*128 + p, m]
    wqkv = nc.dram_tensor("wqkv", [128, 8 * 3 * HL * D], mdt,
                          kind="ExternalInput").ap()
    # host-permuted: wp[p, kc*1024 + o] = W_proj[kc*128 + p, o]
    wp = nc.dram_tensor("wp", [128, 8 * C], mdt,
                        kind="ExternalInput").ap()
    bias = nc.dram_tensor("bias", [128, NCORES], f32,
                          kind="ExternalInput").ap()
    # cols 0:128 identity; cols 128:256 additive causal mask
    cmask = nc.dram_tensor("cmask", [128, 256], mdt,
                           kind="ExternalInput").ap()
    outT = nc.dram_tensor("outT", [C, 512], f32,
                          kind="ExternalOutput").ap()

    causal = mode == "causal"
    Exp = mybir.ActivationFunctionType.Exp
    xh5 = xh.rearrange("p (g nl a u) -> p g nl a u", g=B, nl=4, a=8)

    with tile.TileContext(nc) as tc, \
         nc.allow_low_precision(reason="bf16 matmul path, tol 2e-2"):
        with tc.tile_pool(name="persist", bufs=1) as persist, \
             tc.tile_pool(name="dram", bufs=1, space="DRAM") as dram:
            q_sb = persist.tile([128, NT], mdt)
            k_sb = persist.tile([128, NT], mdt)
            # V^T tiles: vboth[p, h, j, 0:64] = V_h d-columns for k-chunk
            # j; vboth[p, h, j, 64:128] = ones (PV output rows 64:128
            # then hold the softmax denominator l).
            vboth = persist.tile([128, HL, B * NK, 128], mdt)
            cm_sb = persist.tile([128, 256], mdt)
            wqkv_sb = persist.tile([128, 8, 3 * HL * D], mdt)
            wp_sb = persist.tile([128, 8, C], mdt)
            bias_sb = persist.tile([128, NCORES], f32)
            a2a_ins = {}
            a2a_outs = {}
            for ck, (_, parts, _c) in CHUNKS.items():
                w = sum(hi - lo for lo, hi in parts) // NCORES
                a2a_ins[ck] = dram.tile([NCORES * 128, w], mdt,
                                        name=f"a2a_in_{ck}")
                a2a_outs[ck] = dram.tile([NCORES * 128, w], mdt,
                                         name=f"a2a_out_{ck}")
            warm_in = dram.tile([NCORES, 16], mdt)
            warm_out = dram.tile([NCORES, 16], mdt)

            # wqkv on the scalar HWDGE ring: parallel to the x loads on
            # the sync ring, faster than the gpsimd SWDGE path
            nc.scalar.dma_start(out=wqkv_sb[:],
                                in_=wqkv.rearrange("p (a n) -> p a n",
                                                   a=8))
            nc.gpsimd.dma_start(out=cm_sb[:], in_=cmask[:])
            nc.gpsimd.dma_start(out=bias_sb[:], in_=bias[:])
            nc.gpsimd.dma_start(out=wp_sb[:],
                                in_=wp.rearrange("p (a n) -> p a n", a=8))
            for h in range(HL):
                nc.vector.memset(vboth[:, h, :, 64:128], 1.0)
            ident = cm_sb[:, 0:128]
            maskadd = cm_sb[:, 128:256]

            # PSUM layout (8 banks):
            #   mm1 (2 banks): qkv chains + transposes + proj accum
            #   sc  (4 banks): paired score tiles [128,1024] x2 in flight
            #   po  (2 banks): po_h0 / po_h1 accumulators
            with tc.tile_pool(name="mm1", bufs=2, space="PSUM") as mm1, \
                 tc.tile_pool(name="sc_psum", bufs=2, space="PSUM") as scp, \
                 tc.tile_pool(name="po_psum", bufs=1, space="PSUM") as pop, \
                 tc.tile_pool(name="xn_pool", bufs=2) as xp, \
                 tc.tile_pool(name="vtmp_pool", bufs=2) as vpool, \
                 tc.tile_pool(name="at_pool", bufs=6) as apool, \
                 tc.tile_pool(name="rb_pool", bufs=2) as rbp, \
                 tc.tile_pool(name="a_pool", bufs=2) as ap_pool, \
                 tc.tile_pool(name="agt_pool", bufs=3) as agp, \
                 tc.tile_pool(name="out_pool", bufs=8) as outp:

                xns = {}

                def xn_dmas(g):
                    """Load x^T for token group g; 8KB contiguous lines
                    per partition.  nl0 is split in two so the first
                    qkv chains can start a little earlier."""
                    xn = xp.tile([128, 4, 8, 512], mdt, tag="xn",
                                 name="xn")
                    xns[g] = xn
                    if g == 0:
                        nc.sync.dma_start(out=xn[:, 0, 0:4, :],
                                          in_=xh5[:, g, 0, 0:4, :])
                        nc.sync.dma_start(out=xn[:, 0, 4:8, :],
                                          in_=xh5[:, g, 0, 4:8, :])
                    else:
                        nc.sync.dma_start(out=xn[:, 0, :, :],
                                          in_=xh5[:, g, 0, :, :])
                    for nl_ in range(1, 4):
                        nc.sync.dma_start(out=xn[:, nl_, :, :],
                                          in_=xh5[:, g, nl_, :, :])

                def qkv_chain(g, nl, m):
                    """One 8-matmul chain: q/k/v (m=0/1/2) for tokens
                    g*2048+nl*512 .. +512."""
                    xn = xns[g]
                    n = g * 4 + nl
                    tok = slice(n * 512, (n + 1) * 512)
                    ps = mm1.tile([128, 512], f32, tag="ps", name="ps")
                    for kc in range(8):
                        nc.tensor.matmul(
                            ps[:],
                            wqkv_sb[:, kc, m * 128:(m + 1) * 128],
                            xn[:, nl, kc, :],
                            start=(kc == 0), stop=(kc == 7))
                    if m == 0:
                        nc.vector.tensor_copy(q_sb[:, tok], ps[:])
                    elif m == 1:
                        nc.vector.tensor_copy(k_sb[:, tok], ps[:])
                    else:
                        vtmp = vpool.tile([128, 512], mdt, tag="vtmp",
                                          name="vtmp")
                        nc.vector.tensor_copy(vtmp[:], ps[:])
                        bb = n // NQ
                        for s in range(4):
                            j = bb * NK + (n % NQ) * 4 + s
                            pt = mm1.tile([128, 128], mdt, tag="ps",
                                          name="pt")
                            nc.tensor.transpose(
                                pt[:], vtmp[:, s * 128:(s + 1) * 128],
                                ident)
                            nc.vector.tensor_copy(
                                vboth[:, :, j, 0:64],
                                pt[:].rearrange("p (h d) -> p h d", h=2))

                def qkv_thunks(g, nl):
                    return [lambda m=m: qkv_chain(g, nl, m)
                            for m in range(3)]

                def attn_qj(b, qj, a_sb, filler=(), fill_from=0):
                    """Score/exp/PV loop + normalization for (b, qj).
                    Pops one filler thunk per ki iteration starting at
                    iteration fill_from; remaining thunks run after."""
                    filler = deque(filler)
                    last_ki = 4 * qj + 3 if causal else NK - 1
                    po0 = pop.tile([128, 512], f32, tag="po0", name="po0")
                    po1 = pop.tile([128, 512], f32, tag="po1", name="po1")
                    pos = [po0, po1]

                    def emit_pv(ki, at, st):
                        vj = b * NK + ki
                        for h in range(2):
                            nc.tensor.matmul(
                                pos[h][:, st:512],
                                vboth[:, h, vj, :],
                                at[:, 512 * h + st:512 * h + 512],
                                start=(ki == 0), stop=(ki == last_ki))

                    # software-pipelined by one stage: scores(ki+1) sit
                    # ahead of PV(ki) in the PE FIFO so the PE never
                    # stalls on exp(ki)
                    pend = None
                    for ki in range(last_ki + 1):
                        diag = causal and ki >= 4 * qj
                        st = (ki - 4 * qj) * 128 if diag else 0
                        kc_ = slice(b * T + ki * 128,
                                    b * T + (ki + 1) * 128)
                        qc = slice(b * T + qj * 512 + st,
                                   b * T + (qj + 1) * 512)
                        sc = scp.tile([128, 1024], f32, tag="sc",
                                      name="sc")
                        nc.tensor.matmul(
                            sc[:, st:512], k_sb[0:64, kc_],
                            q_sb[0:64, qc], start=True, stop=not diag)
                        nc.tensor.matmul(
                            sc[:, 512 + st:1024], k_sb[64:128, kc_],
                            q_sb[64:128, qc], start=True, stop=not diag)
                        if diag:
                            nc.tensor.matmul(
                                sc[:, st:st + 128], maskadd, ident,
                                start=False, stop=True)
                            nc.tensor.matmul(
                                sc[:, 512 + st:512 + st + 128], maskadd,
                                ident, start=False, stop=True)
                        if pend is not None:
                            emit_pv(*pend)
                        if filler and ki >= fill_from:
                            filler.popleft()()
                        at = apool.tile([128, 1024], mdt, tag="at",
                                        name="at")
                        if st == 0:
                            nc.scalar.activation(at[:], sc[:], Exp,
                                                 scale=SCALE)
                        elif st <= 256:
                            for h in range(2):
                                nc.scalar.activation(
                                    at[:, 512 * h + st:512 * h + 512],
                                    sc[:, 512 * h + st:512 * h + 512],
                                    Exp, scale=SCALE)
                        else:
                            at3 = at[:].rearrange(
                                "p (c t) -> p c t", c=2)[:, :, st:512]
                            sc3 = sc[:].rearrange(
                                "p (c t) -> p c t", c=2)[:, :, st:512]
                            nc.scalar.activation(at3, sc3, Exp,
                                                 scale=SCALE)
                        pend = (ki, at, st)
                    emit_pv(*pend)
                    # normalize: po rows 64:128 are l replicated
                    for h in range(2):
                        lrep = rbp.tile([64, 512], f32, tag="lrep",
                                        name="lrep")
                        nc.vector.tensor_copy(lrep[:], pos[h][64:128, :])
                        rb = rbp.tile([64, 512], f32, tag="rb", name="rb")
                        nc.vector.reciprocal_approx_fast(rb[:], lrep[:])
                        nc.vector.tensor_mul(
                            a_sb[64 * h:64 * h + 64,
                                 qj * 512:(qj + 1) * 512],
                            pos[h][0:64, :], rb[:])
                    while filler:
                        filler.popleft()()

                def stage_part(ck, pi, a_sb):
                    _, parts, _c = CHUNKS[ck]
                    off = sum(h - l for l, h in parts[:pi]) // NCORES
                    lo, hi = parts[pi]
                    pw = (hi - lo) // NCORES
                    inv = a2a_ins[ck][:].rearrange("(c p) t -> p c t",
                                                   p=128)
                    nc.sync.dma_start(
                        out=inv[:, :, off:off + pw],
                        in_=a_sb[:, lo:hi].rearrange("p (c t) -> p c t",
                                                     c=NCORES))

                def a2a(ck):
                    nc.gpsimd.collective_compute(
                        "AllToAll", mybir.AluOpType.bypass,
                        replica_groups=[list(range(NCORES))],
                        ins=[a2a_ins[ck].opt()], outs=[a2a_outs[ck].opt()])

                agts = {}

                def agt_load(ck):
                    _, parts, _c = CHUNKS[ck]
                    w = sum(hi - lo for lo, hi in parts) // NCORES
                    agt = agp.tile([128, 8, w], mdt, tag="agt",
                                   name="agt")
                    agts[ck] = agt
                    nc.gpsimd.dma_start(
                        out=agt[:],
                        in_=a2a_outs[ck][:].rearrange("(c p) t -> p c t",
                                                      p=128))

                def proj_o(ck, o):
                    """One output tile of the local projection for this
                    core's token chunk ck."""
                    _, parts, col = CHUNKS[ck]
                    w = sum(hi - lo for lo, hi in parts) // NCORES
                    agt = agts[ck]
                    pr = mm1.tile([128, w], f32, tag="ps", name="pr")
                    for kc in range(8):
                        nc.tensor.matmul(
                            pr[:], wp_sb[:, kc, o * 128:(o + 1) * 128],
                            agt[:, kc, :],
                            start=(kc == 0), stop=(kc == 7))
                    ot = outp.tile([128, w], f32, tag="ot", name="ot")
                    nc.vector.tensor_scalar_add(ot[:], pr[:],
                                                bias_sb[:, o:o + 1])
                    nc.sync.dma_start(
                        out=outT[o * 128:(o + 1) * 128, col:col + w],
                        in_=ot[:])

                def proj_thunks(ck):
                    return [lambda o=o: proj_o(ck, o) for o in range(8)]

                # tiny warmup collective to absorb first-op CC latency
                nc.gpsimd.collective_compute(
                    "AllToAll", mybir.AluOpType.bypass,
                    replica_groups=[list(range(NCORES))],
                    ins=[warm_in.opt()], outs=[warm_out.opt()])
                a_sb0 = ap_pool.tile([128, T], mdt, tag="a_sb",
                                     name="a_sb0")
                a_sb1 = ap_pool.tile([128, T], mdt, tag="a_sb",
                                     name="a_sb1")
                xn_dmas(0)
                for m in range(3):
                    qkv_chain(0, 0, m)
                attn_qj(0, 0, a_sb0)
                for m in range(3):
                    qkv_chain(0, 1, m)
                attn_qj(0, 1, a_sb0, qkv_thunks(0, 2))
                stage_part("b0h0", 0, a_sb0)
                a2a("b0h0")
                xn_dmas(1)
                attn_qj(0, 2, a_sb0, qkv_thunks(0, 3) + qkv_thunks(1, 0))
                attn_qj(0, 3, a_sb0, qkv_thunks(1, 1) + qkv_thunks(1, 2))
                stage_part("b0h1", 0, a_sb0)
                a2a("b0h1")
                agt_load("b0h0")
                # b1 in order qj1, qj2, qj3, qj0 — per-qj chunks keep the
                # AllToAll pipeline flowing under attention; the cheapest
                # causal block (qj0) runs last so the tail op is small.
                attn_qj(1, 1, a_sb1,
                        qkv_thunks(1, 3) + proj_thunks("b0h0"),
                        fill_from=2)
                stage_part("b1q1", 0, a_sb1)
                a2a("b1q1")
                agt_load("b0h1")
                attn_qj(1, 2, a_sb1, proj_thunks("b0h1"), fill_from=2)
                stage_part("b1q2", 0, a_sb1)
                a2a("b1q2")
                agt_load("b1q1")
                attn_qj(1, 3, a_sb1, proj_thunks("b1q1"), fill_from=4)
                stage_part("b1q3", 0, a_sb1)
                a2a("b1q3")
                agt_load("b1q2")
                attn_qj(1, 0, a_sb1, proj_thunks("b1q2"), fill_from=1)
                stage_part("b1q0", 0, a_sb1)
                a2a("b1q0")
                agt_load("b1q3")
                for o in range(8):
                    proj_o("b1q3", o)
                agt_load("b1q0")
                for o in range(8):
                    proj_o("b1q0", o)

    nc.compile()
    return nc


def _get_program(mode: str):
    if mode not in _cache:
        _cache[mode] = _build(mode)
    return _cache[mode]


def kernel(**inputs):
    import ml_dtypes
    from concourse.bass_utils import run_bass_kernel_spmd

    bf16 = ml_dtypes.bfloat16

    x = np.asarray(inputs["x"], dtype=np.float32)
    mask = np.asarray(inputs["causal_mask"])
    Wqkv = np.asarray(inputs["W_qkv"], dtype=np.float32)
    Wp = np.asarray(inputs["W_proj"], dtype=np.float32)
    bp = np.asarray(inputs["b_proj"], dtype=np.float32)

    m2 = mask.reshape(T, T)
    if np.all(m2 != 0):
        mode = "none"
    else:
        tril = np.tril(np.ones((T, T), dtype=m2.dtype))
        if np.array_equal(m2, tril):
            mode = "causal"
        else:
            raise NotImplementedError("general mask not supported")

    nc = _get_program(mode)

    # xh[p, g, nl, a, u] = x^T[a*128+p, g*2048+nl*512+u]
    xT = np.ascontiguousarray(x.reshape(NT, C).T)           # [1024, 4096]
    xh = np.ascontiguousarray(
        xT.reshape(8, 128, B, 4, 512).transpose(1, 2, 3, 0, 4)
        .reshape(128, B * 4 * 8 * 512)).astype(bf16)

    # [128,256]: identity | additive causal mask (-1e5 above diagonal)
    p = np.arange(128)[:, None]
    f = np.arange(128)[None, :]
    madd = np.where(p < f, MASKNEG, 0.0).astype(np.float32)
    cm = np.concatenate(
        [np.eye(128, dtype=np.float32), madd], axis=1).astype(bf16)

    Wq = Wqkv[:, 0 * C:1 * C]
    Wk = Wqkv[:, 1 * C:2 * C]
    Wv = Wqkv[:, 2 * C:3 * C]
    wp_bf = np.ascontiguousarray(
        Wp.reshape(8, 128, C).transpose(1, 0, 2).reshape(128, 8 * C)
    ).astype(bf16)
    bias_h = np.ascontiguousarray(bp.reshape(NCORES, 128).T)

    in_maps = []
    for i in range(NCORES):
        hcols = slice(2 * i * D, (2 * i + 2) * D)  # this core's 2 heads
        wqkv_i = np.concatenate(
            [Wq[:, hcols], Wk[:, hcols], Wv[:, hcols]], axis=1)
        wqkv_p = np.ascontiguousarray(
            wqkv_i.reshape(8, 128, 3 * HL * D).transpose(1, 0, 2)
            .reshape(128, 8 * 3 * HL * D)).astype(bf16)
        in_maps.append({
            "xh": xh,
            "wqkv": wqkv_p,
            "wp": wp_bf,
            "bias": bias_h,
            "cmask": cm,
        })

    res = run_bass_kernel_spmd(nc, in_maps, list(range(NCORES)))

    out = np.empty((B, T, C), dtype=np.float32)
    for i in range(NCORES):
        oT = res.results[i]["outT"]  # [C, 512] f32
        for _ck, (b, parts, col) in CHUNKS.items():
            c = col
            for lo, hi in parts:
                w = (hi - lo) // NCORES
                out[b, lo + i * w:lo + (i + 1) * w, :] = \
                    oT[:, c:c + w].T
                c += w
    return out
